# revision 21
# baseline (speedup 1.0000x reference)
"""CrossAttention on 8 TRN2 NeuronCores (tensor-parallel over heads).

Reference computation (B=4, N=2048, DIM=1024, 16 heads, head_dim=64):
    qkv = x @ Wqkv.T + bqkv ; q, k = split(qkv)  (v unused)
    attn = softmax(q @ k.T * scale) ; out = attn @ split_heads(context)
    return merge_heads(out) @ Wout.T + bout

Sharding: core c owns heads {2c, 2c+1}. Each core computes q/k
projections for its heads (full sequence), head-parallel attention with
context slices as values, then a per-batch AllToAll re-shards from
head-parallel to row-parallel so the output projection runs locally.
Row ownership is interleaved (core c owns rows [c*256:(c+1)*256] of
every batch).

Steady state is ScalarE-bound (exp of the full attention matrix, one
[128,1024] ACTIVATE per key-chunk). The emission interleaves the q/k
projection of batch b+1 and the output projection of batch b-1 as small
matmul units inside attention(b)'s key-chunk loop so neither TensorE
nor ScalarE ever starves and the PE stays HAM-warm to the end.
"""
import numpy as np
import ml_dtypes

import concourse.bass as bass
import concourse.mybir as mybir
import concourse.tile as tile
from concourse import bacc
from concourse.bass_utils import run_bass_kernel_spmd

BF16 = ml_dtypes.bfloat16
F32 = mybir.dt.float32
BF = mybir.dt.bfloat16

NC = 8            # cores
B = 4             # batch
N = 2048          # sequence
DIM = 1024
NH = 16           # heads total
HD = 64           # head dim
HPC = NH // NC    # heads per core = 2
SCALE = HD ** -0.5
BN = B * N        # 8192 tokens
RPB = N // NC     # rows per (core, batch) after re-shard = 256
KC = DIM // 128   # contraction chunks for projections = 8
NKC = N // 128    # key chunks per batch = 16
CW = HD + 1       # value width incl. ones column = 65
NT = 4            # 512-token chunks per batch
PT_BUFS = 6


def build():
    nc = bacc.Bacc("TRN2", target_bir_lowering=False, debug=False,
                   num_devices=NC)

    xT = nc.dram_tensor("xT", [DIM, BN], BF, kind="ExternalInput")
    wqkT = nc.dram_tensor("wqkT", [DIM, 2 * 128], BF, kind="ExternalInput")
    bqk = nc.dram_tensor("bqk", [2 * 128, 1], F32, kind="ExternalInput")
    ctxa = nc.dram_tensor("ctxa", [B, HPC, 128, NKC * CW], BF,
                          kind="ExternalInput")
    woutT = nc.dram_tensor("woutT", [DIM, DIM], BF, kind="ExternalInput")
    boutb = nc.dram_tensor("boutb", [128, DIM], F32, kind="ExternalInput")
    # out rows: batch-major, 256 rows per batch (this core's rows)
    out = nc.dram_tensor("out", [B * RPB, DIM], F32, kind="ExternalOutput")

    # per-(batch, half) AllToAll bounce buffers; in half hf, chunk j
    # holds rows [hf*1024 + j*128 : hf*1024 + (j+1)*128] of batch b and
    # is delivered to core j. Core c therefore owns two 128-row stripes
    # per batch: hf*1024 + c*128 for hf in {0, 1}.
    a2a_in = [[nc.dram_tensor(f"a2a_in{b}_{hf}", [NC, 128, 128], BF)
               for hf in range(2)] for b in range(B)]
    a2a_out = [[nc.dram_tensor(f"a2a_out{b}_{hf}", [NC, 128, 128], BF)
                for hf in range(2)] for b in range(B)]



    with tile.TileContext(nc) as tc:
        with tc.tile_pool(name="const", bufs=1) as const, \
             tc.tile_pool(name="qk", bufs=1) as qkpool, \
             tc.tile_pool(name="xt", bufs=40) as xtpool, \
             tc.tile_pool(name="pt", bufs=PT_BUFS) as ptpool, \
             tc.tile_pool(name="r1", bufs=4) as r1pool, \
             tc.tile_pool(name="ho", bufs=4) as hopool, \
             tc.tile_pool(name="sl", bufs=16) as slpool, \
             tc.tile_pool(name="ob", bufs=4) as obpool, \
             tc.tile_pool(name="pc", bufs=4) as pcpool, \
             tc.tile_pool(name="pss", bufs=2, space="PSUM") as pss_pool, \
             tc.tile_pool(name="pav", bufs=2, space="PSUM") as pav_pool, \
             tc.tile_pool(name="pj", bufs=2, space="PSUM") as pj_pool:

            # ---- small constants needed up front ----
            wqk_sb = []
            for kc in range(KC):
                t = const.tile([128, 256], BF, tag=f"wqk{kc}")
                nc.sync.dma_start(out=t[:], in_=wqkT[kc * 128:(kc + 1) * 128, :])
                wqk_sb.append(t)
            bq_sb = []
            for fb in range(2):
                t = const.tile([128, 1], F32, tag=f"bq{fb}")
                nc.sync.dma_start(out=t[:], in_=bqk[fb * 128:(fb + 1) * 128, :])
                bq_sb.append(t)
            # ones row for the K=1 broadcast matmul in softmax normalize
            ones_sb = const.tile([1, HD], F32, tag="ones")
            nc.vector.memset(ones_sb[:], 1.0)
            # warm the exp activation-table set before it is on the
            # critical path (ACT_TABLE_LOAD is ~2.7us)
            tl = r1pool.tile([128, 1], F32, tag="tblwarm", name="tblwarm")
            nc.scalar.activation(tl[:], bq_sb[0][:],
                                 mybir.ActivationFunctionType.Exp)

            wout_sb = []
            bout_sb = const.tile([128, DIM], F32, tag="bout")
            ctx_sb = {}
            qk_tiles = {}
            xt_tiles = {}

            def load_out_consts():
                for fc in range(KC):
                    t = const.tile([128, DIM], BF, tag=f"wout{fc}",
                                   name=f"wout{fc}")
                    nc.sync.dma_start(
                        out=t[:], in_=woutT[fc * 128:(fc + 1) * 128, :])
                    wout_sb.append(t)
                nc.sync.dma_start(out=bout_sb[:], in_=boutb[:])

            def load_ctx(b):
                for h in range(HPC):
                    t = const.tile([128, NKC * CW], BF, tag=f"ctx{b % 2}{h}",
                                   name=f"ctx{b}_{h}")
                    nc.sync.dma_start(out=t[:], in_=ctxa[b, h, :, :])
                    ctx_sb[b, h] = t

            def prefetch_x(b):
                """Issue per-(kc, t) xT DMAs and allocate q/k for batch b."""
                qT = qkpool.tile([128, N], BF, tag=f"qT{b % 2}", name=f"qT{b}")
                kT = qkpool.tile([128, N], BF, tag=f"kT{b % 2}", name=f"kT{b}")
                qk_tiles[b] = (qT, kT)
                for t in range(NT):
                    for kc in range(KC):
                        xt = xtpool.tile([128, 512], BF, tag="xt",
                                         name=f"xtb{b}_{kc}_{t}")
                        nc.sync.dma_start(
                            out=xt[:],
                            in_=xT[kc * 128:(kc + 1) * 128,
                                   b * N + t * 512:b * N + (t + 1) * 512])
                        xt_tiles[b, kc, t] = xt

            def qkproj_unit(b, t, fb):
                """Project one (512-token, q-or-k) slice of batch b."""
                qT, kT = qk_tiles[b]
                dst = kT if fb == 1 else qT
                ps = pj_pool.tile([128, 512], F32, tag="pj",
                                  name=f"psq{b}_{t}_{fb}")
                for kc in range(KC):
                    nc.tensor.matmul(
                        ps[:], wqk_sb[kc][:, fb * 128:(fb + 1) * 128],
                        xt_tiles[b, kc, t][:],
                        start=(kc == 0), stop=(kc == KC - 1))
                nc.vector.tensor_scalar_add(
                    dst[:, t * 512:(t + 1) * 512], ps[:], bq_sb[fb][:])

            def outproj_unit(b, rc, n):
                """512 output features (n) for row-stripe rc of batch b."""
                if n == 0:
                    sls = []
                    for fc in range(KC):
                        sl = slpool.tile([128, 128], BF, tag="sl",
                                         name=f"sl{b}_{rc}_{fc}")
                        nc.sync.dma_start(
                            out=sl[:], in_=a2a_out[b][rc][fc, :, :])
                        sls.append(sl)
                    outproj_unit.sls[b, rc] = sls
                sls = outproj_unit.sls[b, rc]
                pso = pj_pool.tile([128, 512], F32, tag="pj",
                                   name=f"pso{b}_{rc}_{n}")
                for fc in range(KC):
                    nc.tensor.matmul(
                        pso[:], sls[fc][:],
                        wout_sb[fc][:, n * 512:(n + 1) * 512],
                        start=(fc == 0), stop=(fc == KC - 1))
                ob = obpool.tile([128, 512], F32, tag="ob",
                                 name=f"ob{b}_{rc}_{n}")
                nc.vector.tensor_tensor(
                    out=ob[:], in0=pso[:],
                    in1=bout_sb[:, n * 512:(n + 1) * 512],
                    op=mybir.AluOpType.add)
                nc.sync.dma_start(
                    out=out[b * RPB + rc * 128:b * RPB + (rc + 1) * 128,
                            n * 512:(n + 1) * 512],
                    in_=ob[:])
                # out row b*256 + rc*128 + i holds batch-b global row
                # rc*1024 + c*128 + i (stripe ownership)
            outproj_unit.sls = {}

            def attention_qg(b, qg, fillers):
                """Both heads' scores+softmax+values for 512 queries.

                fillers: dict kc -> list of thunks emitted after that
                key-chunk's exp (projection work woven into the stream).
                """
                qT, kT = qk_tiles[b]
                q0 = qg * 512
                pts = []
                for kc in range(NKC):
                    ps = pss_pool.tile([128, 1024], F32, tag="pss",
                                       name=f"pss{b}{qg}{kc}")
                    for h in range(HPC):
                        nc.tensor.matmul(
                            ps[:, h * 512:(h + 1) * 512],
                            kT[h * HD:(h + 1) * HD, kc * 128:(kc + 1) * 128],
                            qT[h * HD:(h + 1) * HD, q0:q0 + 512],
                            start=True, stop=True,
                            tile_position=(h * HD, 0))
                    pt = ptpool.tile([128, 1024], BF, tag="pt",
                                     name=f"pt{b}_{qg}_{kc}")
                    nc.scalar.activation(
                        pt[:], ps[:],
                        mybir.ActivationFunctionType.Exp, scale=SCALE)
                    pts.append(pt)
                    for f in fillers.get(kc, ()):
                        f()
                # stage-major normalize: PSUM evictions first (frees pav
                # slots for the next group's chains), then reciprocals.
                # The returned closure finishes later — an on-chip K=1
                # matmul broadcasts 1/den across the 64 head-dim
                # partitions (no DRAM round-trip) — woven into the NEXT
                # group's stream so the slow reciprocal never stalls PE.
                pcs, r1s = [], []
                for h in range(HPC):
                    pav = pav_pool.tile([CW, 512], F32, tag="pav",
                                        name=f"pav{b}{qg}{h}")
                    for kc in range(NKC):
                        nc.tensor.matmul(
                            pav[:], ctx_sb[b, h][:, kc * CW:(kc + 1) * CW],
                            pts[kc][:, h * 512:(h + 1) * 512],
                            start=(kc == 0), stop=(kc == NKC - 1))
                    pc = pcpool.tile([CW, 512], F32, tag="pc",
                                     name=f"pc{b}{qg}{h}")
                    nc.vector.tensor_copy(pc[:], pav[:])
                    pcs.append(pc)
                for h in range(HPC):
                    r1 = r1pool.tile([1, 512], F32, tag="r1",
                                     name=f"r1{b}{qg}{h}")
                    nc.vector.reciprocal(r1[:], pcs[h][HD:CW, :])
                    r1s.append(r1)

                def finish(b=b, qg=qg, q0=q0, pcs=pcs, r1s=r1s):
                    for h in range(HPC):
                        rb = pav_pool.tile([HD, 512], F32, tag="pav",
                                           name=f"rb{b}{qg}{h}")
                        nc.tensor.matmul(rb[:], ones_sb[:], r1s[h][:],
                                         start=True, stop=True)
                        ho = hopool.tile([HD, 512], BF, tag="ho",
                                         name=f"ho{b}{qg}{h}")
                        nc.vector.tensor_tensor(
                            out=ho[:], in0=pcs[h][0:HD, :], in1=rb[:],
                            op=mybir.AluOpType.mult)
                        for m in range(4):
                            row = q0 + m * 128
                            hf, j = row // 1024, (row % 1024) // 128
                            nc.sync.dma_start(
                                out=a2a_in[b][hf][j, h * HD:(h + 1) * HD, :],
                                in_=ho[:, m * 128:(m + 1) * 128])
                return finish

            def reshard(b, hf):
                nc.gpsimd.collective_compute(
                    "AllToAll", mybir.AluOpType.bypass,
                    replica_groups=[list(range(NC))],
                    ins=[a2a_in[b][hf].ap().opt()],
                    outs=[a2a_out[b][hf].ap().opt()])

            # ---------------- emission ----------------
            prefetch_x(0)
            load_ctx(0)
            # scores(qg0, kc) only needs k of token-chunk kc//4 and q of
            # chunk 0 — project those, then start attention immediately
            # and weave the remaining five projection slices into qg0
            qkproj_unit(0, 0, 1)
            qkproj_unit(0, 0, 0)
            prefetch_x(1)
            load_ctx(1)
            load_out_consts()

            pending = []  # deferred normalize-finish of the previous group
            for b in range(B):
                for qg in range(4):
                    fillers = {}
                    if pending:
                        fillers.setdefault(6, []).append(pending.pop())
                    if b == 0 and qg == 0:
                        # remaining k/q-projection slices of b0: k(t)
                        # must land before scores reach kc = 4*t
                        for slot, t, fb in ((0, 1, 1), (1, 2, 1), (2, 3, 1),
                                            (3, 1, 0), (5, 2, 0), (7, 3, 0)):
                            fillers.setdefault(slot, []).append(
                                lambda t=t, fb=fb: qkproj_unit(0, t, fb))
                    if b + 1 < B:
                        # q/k projection of the next batch: 8 units per
                        # batch (shifted late in qg0 of b0 to stay off the
                        # warm-up critical path)
                        t = qg
                        s1, s2 = (11, 13) if b == 0 and qg == 0 else (1, 9)
                        fillers.setdefault(s1, []).append(
                            lambda b=b, t=t: qkproj_unit(b + 1, t, 1))
                        fillers.setdefault(s2, []).append(
                            lambda b=b, t=t: qkproj_unit(b + 1, t, 0))
                    if b >= 1 and qg >= 2:
                        # output projection of the previous batch in the
                        # second half of this batch's attention, so its
                        # AllToAll has certainly landed
                        rc = qg - 2
                        fillers.setdefault(5, []).append(
                            lambda b=b, rc=rc: outproj_unit(b - 1, rc, 0))
                        fillers.setdefault(13, []).append(
                            lambda b=b, rc=rc: outproj_unit(b - 1, rc, 1))
                    fin = attention_qg(b, qg, fillers)
                    if qg == 3:
                        # last group of the batch: finish inline so the
                        # second-half collective can be emitted now (its
                        # dependency set must include these DMAs)
                        fin()
                    else:
                        pending.append(fin)
                    if qg == 2:
                        # both first-half finishes (qg0 in qg1, qg1 here)
                        # have been emitted by now
                        reshard(b, 0)
                        if b + 2 < B:
                            prefetch_x(b + 2)
                            load_ctx(b + 2)
                reshard(b, 1)
            # tail: outproj(3, rc0)'s a2a landed mid-batch; rc1's flies
            # under rc0's matmuls
            for rc in range(2):
                for n in range(2):
                    outproj_unit(3, rc, n)
    nc.compile()
    return nc


def prep_inputs(x, context, Wqkv, bqkv, Wout, bout):
    """Host-side sharding: returns in_maps for the 8 cores."""
    x = np.asarray(x, np.float32)
    context = np.asarray(context, np.float32)
    Wqkv = np.asarray(Wqkv, np.float32)
    bqkv = np.asarray(bqkv, np.float32)
    Wout = np.asarray(Wout, np.float32)
    bout = np.asarray(bout, np.float32)

    xT = np.ascontiguousarray(x.reshape(BN, DIM).T).astype(BF16)
    woutT = np.ascontiguousarray(Wout.T).astype(BF16)
    boutb = np.broadcast_to(bout, (128, DIM)).astype(np.float32).copy()

    in_maps = []
    for c in range(NC):
        h0 = c * HPC
        # feature order: [q_h0 | q_h1] then [k_h0 | k_h1]
        wq = Wqkv[h0 * HD:(h0 + HPC) * HD]
        wk = Wqkv[DIM + h0 * HD:DIM + (h0 + HPC) * HD]
        wqkT = np.ascontiguousarray(
            np.concatenate([wq, wk], axis=0).T).astype(BF16)
        bq = np.concatenate([bqkv[h0 * HD:(h0 + HPC) * HD],
                             bqkv[DIM + h0 * HD:DIM + (h0 + HPC) * HD]])
        bq = bq.reshape(2 * 128, 1).astype(np.float32)
        ctxa = np.ones((B, HPC, 128, NKC, CW), np.float32)
        for h in range(HPC):
            g = h0 + h
            arr = context[:, :, g * HD:(g + 1) * HD].reshape(B, NKC, 128, HD)
            ctxa[:, h, :, :, :HD] = arr.transpose(0, 2, 1, 3)
        in_maps.append({
            "xT": xT,
            "wqkT": wqkT,
            "bqk": bq,
            "ctxa": ctxa.reshape(B, HPC, 128, NKC * CW).astype(BF16),
            "woutT": woutT,
            "boutb": boutb,
        })
    return in_maps


_NC_CACHE = None


def _get_nc():
    global _NC_CACHE
    if _NC_CACHE is None:
        _NC_CACHE = build()
    return _NC_CACHE


def run(in_maps, trace=False):
    nc = _get_nc()
    res = run_bass_kernel_spmd(nc, in_maps, core_ids=list(range(NC)),
                               trace=trace)
    # core c's out = [B, 2, 128, DIM]: stripe (b, hf) holds batch-b rows
    # [hf*1024 + c*128 : hf*1024 + (c+1)*128]
    full = np.empty((B, N, DIM), np.float32)
    for c in range(NC):
        o = np.asarray(res.results[c]["out"]).reshape(B, 2, 128, DIM)
        for hf in range(2):
            full[:, hf * 1024 + c * 128:hf * 1024 + (c + 1) * 128, :] = \
                o[:, hf]
    return full, res


def kernel(x, context, Wqkv, bqkv, Wout, bout):
    in_maps = prep_inputs(x, context, Wqkv, bqkv, Wout, bout)
    out, _ = run(in_maps, trace=False)
    return out


# revision 25
# speedup vs baseline: 1.3210x; 1.3210x over previous
"""CrossAttention on 8 TRN2 NeuronCores (tensor-parallel over heads).

Reference computation (B=4, N=2048, DIM=1024, 16 heads, head_dim=64):
    qkv = x @ Wqkv.T + bqkv ; q, k = split(qkv)  (v unused)
    attn = softmax(q @ k.T * scale) ; out = attn @ split_heads(context)
    return merge_heads(out) @ Wout.T + bout

Sharding: core c owns heads {2c, 2c+1}. Each core computes q/k
projections for its heads (full sequence), head-parallel attention with
context slices as values, then a per-batch AllToAll re-shards from
head-parallel to row-parallel so the output projection runs locally.
Row ownership is interleaved (core c owns rows [c*256:(c+1)*256] of
every batch).

Steady state is ScalarE-bound (exp of the full attention matrix, one
[128,1024] ACTIVATE per key-chunk). The emission interleaves the q/k
projection of batch b+1 and the output projection of batch b-1 as small
matmul units inside attention(b)'s key-chunk loop so neither TensorE
nor ScalarE ever starves and the PE stays HAM-warm to the end.
"""
import numpy as np
import ml_dtypes

import concourse.bass as bass
import concourse.mybir as mybir
import concourse.tile as tile
from concourse import bacc
from concourse.bass_utils import run_bass_kernel_spmd

BF16 = ml_dtypes.bfloat16
F32 = mybir.dt.float32
BF = mybir.dt.bfloat16

NC = 8            # cores
B = 4             # batch
N = 2048          # sequence
DIM = 1024
NH = 16           # heads total
HD = 64           # head dim
HPC = NH // NC    # heads per core = 2
SCALE = HD ** -0.5
BN = B * N        # 8192 tokens
RPB = N // NC     # rows per (core, batch) after re-shard = 256
KC = DIM // 128   # contraction chunks for projections = 8
NKC = N // 128    # key chunks per batch = 16
CW = HD + 1       # value width incl. ones column = 65
NT = 4            # 512-token chunks per batch
PT_BUFS = 6


def build():
    nc = bacc.Bacc("TRN2", target_bir_lowering=False, debug=False,
                   num_devices=NC)

    xT = nc.dram_tensor("xT", [DIM, BN], BF, kind="ExternalInput")
    wqkT = nc.dram_tensor("wqkT", [DIM, 2 * 128], BF, kind="ExternalInput")
    bqk = nc.dram_tensor("bqk", [2 * 128, 1], F32, kind="ExternalInput")
    ctxa = nc.dram_tensor("ctxa", [B, HPC, 128, NKC * CW], BF,
                          kind="ExternalInput")
    woutT = nc.dram_tensor("woutT", [DIM, DIM], BF, kind="ExternalInput")
    boutb = nc.dram_tensor("boutb", [128, DIM], F32, kind="ExternalInput")
    # out rows: batch-major, 256 rows per batch (this core's rows)
    out = nc.dram_tensor("out", [B * RPB, DIM], F32, kind="ExternalOutput")

    # per-(batch, half) AllToAll bounce buffers; in half hf, chunk j
    # holds rows [hf*1024 + j*128 : hf*1024 + (j+1)*128] of batch b and
    # is delivered to core j. Core c therefore owns two 128-row stripes
    # per batch: hf*1024 + c*128 for hf in {0, 1}.
    a2a_in = [[nc.dram_tensor(f"a2a_in{b}_{hf}", [NC, 128, 128], BF)
               for hf in range(2)] for b in range(B)]
    a2a_out = [[nc.dram_tensor(f"a2a_out{b}_{hf}", [NC, 128, 128], BF)
                for hf in range(2)] for b in range(B)]

    rscr = [nc.dram_tensor(f"rscr{i}", [1, 512], F32) for i in range(16)]
    _scr_idx = [0]



    with tile.TileContext(nc) as tc:
        with tc.tile_pool(name="const", bufs=1) as const, \
             tc.tile_pool(name="qk", bufs=1) as qkpool, \
             tc.tile_pool(name="xt", bufs=40) as xtpool, \
             tc.tile_pool(name="pt", bufs=PT_BUFS) as ptpool, \
             tc.tile_pool(name="r1", bufs=4) as r1pool, \
             tc.tile_pool(name="rb", bufs=4) as rbpool, \
             tc.tile_pool(name="ho", bufs=4) as hopool, \
             tc.tile_pool(name="sl", bufs=16) as slpool, \
             tc.tile_pool(name="ob", bufs=4) as obpool, \
             tc.tile_pool(name="pc", bufs=4) as pcpool, \
             tc.tile_pool(name="pss", bufs=2, space="PSUM") as pss_pool, \
             tc.tile_pool(name="pav", bufs=2, space="PSUM") as pav_pool, \
             tc.tile_pool(name="pj", bufs=2, space="PSUM") as pj_pool:

            # ---- small constants needed up front ----
            wqk_sb = []
            for kc in range(KC):
                t = const.tile([128, 256], BF, tag=f"wqk{kc}")
                nc.sync.dma_start(out=t[:], in_=wqkT[kc * 128:(kc + 1) * 128, :])
                wqk_sb.append(t)
            bq_sb = []
            for fb in range(2):
                t = const.tile([128, 1], F32, tag=f"bq{fb}")
                nc.sync.dma_start(out=t[:], in_=bqk[fb * 128:(fb + 1) * 128, :])
                bq_sb.append(t)
            # warm the exp activation-table set before it is on the
            # critical path (ACT_TABLE_LOAD is ~2.7us)
            tl = r1pool.tile([128, 1], F32, tag="tblwarm", name="tblwarm")
            nc.scalar.activation(tl[:], bq_sb[0][:],
                                 mybir.ActivationFunctionType.Exp)

            wout_sb = []
            bout_sb = const.tile([128, DIM], F32, tag="bout")
            ctx_sb = {}
            qk_tiles = {}
            xt_tiles = {}

            def load_out_consts():
                for fc in range(KC):
                    t = const.tile([128, DIM], BF, tag=f"wout{fc}",
                                   name=f"wout{fc}")
                    nc.sync.dma_start(
                        out=t[:], in_=woutT[fc * 128:(fc + 1) * 128, :])
                    wout_sb.append(t)
                nc.sync.dma_start(out=bout_sb[:], in_=boutb[:])

            def load_ctx(b):
                for h in range(HPC):
                    t = const.tile([128, NKC * CW], BF, tag=f"ctx{b % 2}{h}",
                                   name=f"ctx{b}_{h}")
                    nc.sync.dma_start(out=t[:], in_=ctxa[b, h, :, :])
                    ctx_sb[b, h] = t

            def prefetch_x(b):
                """Issue per-(kc, t) xT DMAs and allocate q/k for batch b."""
                qT = qkpool.tile([128, N], BF, tag=f"qT{b % 2}", name=f"qT{b}")
                kT = qkpool.tile([128, N], BF, tag=f"kT{b % 2}", name=f"kT{b}")
                qk_tiles[b] = (qT, kT)
                for t in range(NT):
                    for kc in range(KC):
                        xt = xtpool.tile([128, 512], BF, tag="xt",
                                         name=f"xtb{b}_{kc}_{t}")
                        nc.sync.dma_start(
                            out=xt[:],
                            in_=xT[kc * 128:(kc + 1) * 128,
                                   b * N + t * 512:b * N + (t + 1) * 512])
                        xt_tiles[b, kc, t] = xt

            def qkproj_unit(b, t, fb):
                """Project one (512-token, q-or-k) slice of batch b."""
                qT, kT = qk_tiles[b]
                dst = kT if fb == 1 else qT
                ps = pj_pool.tile([128, 512], F32, tag="pj",
                                  name=f"psq{b}_{t}_{fb}")
                for kc in range(KC):
                    nc.tensor.matmul(
                        ps[:], wqk_sb[kc][:, fb * 128:(fb + 1) * 128],
                        xt_tiles[b, kc, t][:],
                        start=(kc == 0), stop=(kc == KC - 1))
                nc.vector.tensor_scalar_add(
                    dst[:, t * 512:(t + 1) * 512], ps[:], bq_sb[fb][:])

            def outproj_unit(b, rc, n):
                """512 output features (n) for row-stripe rc of batch b."""
                if n == 0:
                    sls = []
                    for fc in range(KC):
                        sl = slpool.tile([128, 128], BF, tag="sl",
                                         name=f"sl{b}_{rc}_{fc}")
                        nc.sync.dma_start(
                            out=sl[:], in_=a2a_out[b][rc][fc, :, :])
                        sls.append(sl)
                    outproj_unit.sls[b, rc] = sls
                sls = outproj_unit.sls[b, rc]
                pso = pj_pool.tile([128, 512], F32, tag="pj",
                                   name=f"pso{b}_{rc}_{n}")
                for fc in range(KC):
                    nc.tensor.matmul(
                        pso[:], sls[fc][:],
                        wout_sb[fc][:, n * 512:(n + 1) * 512],
                        start=(fc == 0), stop=(fc == KC - 1))
                ob = obpool.tile([128, 512], F32, tag="ob",
                                 name=f"ob{b}_{rc}_{n}")
                nc.vector.tensor_tensor(
                    out=ob[:], in0=pso[:],
                    in1=bout_sb[:, n * 512:(n + 1) * 512],
                    op=mybir.AluOpType.add)
                nc.sync.dma_start(
                    out=out[b * RPB + rc * 128:b * RPB + (rc + 1) * 128,
                            n * 512:(n + 1) * 512],
                    in_=ob[:])
                # out row b*256 + rc*128 + i holds batch-b global row
                # rc*1024 + c*128 + i (stripe ownership)
            outproj_unit.sls = {}

            def attention_qg(b, qg, fillers):
                """Both heads' scores+softmax+values for 512 queries.

                fillers: dict kc -> list of thunks emitted after that
                key-chunk's exp (projection work woven into the stream).
                """
                qT, kT = qk_tiles[b]
                q0 = qg * 512
                pts = []
                for kc in range(NKC):
                    ps = pss_pool.tile([128, 1024], F32, tag="pss",
                                       name=f"pss{b}{qg}{kc}")
                    for h in range(HPC):
                        nc.tensor.matmul(
                            ps[:, h * 512:(h + 1) * 512],
                            kT[h * HD:(h + 1) * HD, kc * 128:(kc + 1) * 128],
                            qT[h * HD:(h + 1) * HD, q0:q0 + 512],
                            start=True, stop=True,
                            tile_position=(h * HD, 0))
                    pt = ptpool.tile([128, 1024], BF, tag="pt",
                                     name=f"pt{b}_{qg}_{kc}")
                    nc.scalar.activation(
                        pt[:], ps[:],
                        mybir.ActivationFunctionType.Exp, scale=SCALE)
                    pts.append(pt)
                    for f in fillers.get(kc, ()):
                        f()
                # stage-major normalize: PSUM evictions first (frees pav
                # slots for the next group's chains), then reciprocals.
                # The returned closure finishes later — an on-chip K=1
                # matmul broadcasts 1/den across the 64 head-dim
                # partitions (no DRAM round-trip) — woven into the NEXT
                # group's stream so the slow reciprocal never stalls PE.
                pcs, r1s = [], []
                for h in range(HPC):
                    pav = pav_pool.tile([CW, 512], F32, tag="pav",
                                        name=f"pav{b}{qg}{h}")
                    for kc in range(NKC):
                        nc.tensor.matmul(
                            pav[:], ctx_sb[b, h][:, kc * CW:(kc + 1) * CW],
                            pts[kc][:, h * 512:(h + 1) * 512],
                            start=(kc == 0), stop=(kc == NKC - 1))
                    pc = pcpool.tile([CW, 512], F32, tag="pc",
                                     name=f"pc{b}{qg}{h}")
                    nc.vector.tensor_copy(pc[:], pav[:])
                    pcs.append(pc)
                for h in range(HPC):
                    r1 = r1pool.tile([1, 512], F32, tag="r1",
                                     name=f"r1{b}{qg}{h}")
                    nc.vector.reciprocal(r1[:], pcs[h][HD:CW, :])
                    r1s.append(r1)

                # broadcast 1/den to 64 partitions via a DRAM round-trip
                # (pure DMA — never touches the PE stream); issue now so
                # the bounce is in flight before finish() consumes it
                rbs = []
                for h in range(HPC):
                    scr = rscr[_scr_idx[0] % 16]; _scr_idx[0] += 1
                    nc.sync.dma_start(out=scr[:], in_=r1s[h][:])
                    rb = rbpool.tile([HD, 512], F32, tag="rb",
                                     name=f"rb{b}{qg}{h}")
                    nc.sync.dma_start(out=rb[:],
                                      in_=scr[:].broadcast_to([HD, 512]))
                    rbs.append(rb)

                def finish(b=b, qg=qg, q0=q0, pcs=pcs, rbs=rbs):
                    for h in range(HPC):
                        ho = hopool.tile([HD, 512], BF, tag="ho",
                                         name=f"ho{b}{qg}{h}")
                        nc.vector.tensor_tensor(
                            out=ho[:], in0=pcs[h][0:HD, :], in1=rbs[h][:],
                            op=mybir.AluOpType.mult)
                        for m in range(4):
                            row = q0 + m * 128
                            hf, j = row // 1024, (row % 1024) // 128
                            nc.sync.dma_start(
                                out=a2a_in[b][hf][j, h * HD:(h + 1) * HD, :],
                                in_=ho[:, m * 128:(m + 1) * 128])
                return finish

            def reshard(b, hf):
                nc.gpsimd.collective_compute(
                    "AllToAll", mybir.AluOpType.bypass,
                    replica_groups=[list(range(NC))],
                    ins=[a2a_in[b][hf].ap().opt()],
                    outs=[a2a_out[b][hf].ap().opt()])

            # ---------------- emission ----------------
            prefetch_x(0)
            load_ctx(0)
            # scores(qg0, kc) only needs k of token-chunk kc//4 and q of
            # chunk 0 — project those, then start attention immediately
            # and weave the remaining five projection slices into qg0
            qkproj_unit(0, 0, 1)
            qkproj_unit(0, 0, 0)
            prefetch_x(1)
            load_ctx(1)
            load_out_consts()

            pending = []  # deferred normalize-finish of the previous group
            for b in range(B):
                for qg in range(4):
                    fillers = {}
                    if pending:
                        fillers.setdefault(6, []).append(pending.pop())
                    if b == 0 and qg == 0:
                        # remaining k/q-projection slices of b0: k(t)
                        # must land before scores reach kc = 4*t
                        for slot, t, fb in ((0, 1, 1), (1, 2, 1), (2, 3, 1),
                                            (3, 1, 0), (5, 2, 0), (7, 3, 0)):
                            fillers.setdefault(slot, []).append(
                                lambda t=t, fb=fb: qkproj_unit(0, t, fb))
                    if b + 1 < B:
                        # q/k projection of the next batch: 8 units per
                        # batch (shifted late in qg0 of b0 to stay off the
                        # warm-up critical path)
                        t = qg
                        s1, s2 = (11, 13) if b == 0 and qg == 0 else (1, 9)
                        fillers.setdefault(s1, []).append(
                            lambda b=b, t=t: qkproj_unit(b + 1, t, 1))
                        fillers.setdefault(s2, []).append(
                            lambda b=b, t=t: qkproj_unit(b + 1, t, 0))
                    if b >= 1 and qg >= 2:
                        # output projection of the previous batch in the
                        # second half of this batch's attention, so its
                        # AllToAll has certainly landed
                        rc = qg - 2
                        fillers.setdefault(5, []).append(
                            lambda b=b, rc=rc: outproj_unit(b - 1, rc, 0))
                        fillers.setdefault(13, []).append(
                            lambda b=b, rc=rc: outproj_unit(b - 1, rc, 1))
                    fin = attention_qg(b, qg, fillers)
                    if qg == 3:
                        # last group of the batch: finish inline so the
                        # second-half collective can be emitted now (its
                        # dependency set must include these DMAs)
                        fin()
                    else:
                        pending.append(fin)
                    if qg == 2:
                        # both first-half finishes (qg0 in qg1, qg1 here)
                        # have been emitted by now
                        reshard(b, 0)
                        if b + 2 < B:
                            prefetch_x(b + 2)
                            load_ctx(b + 2)
                reshard(b, 1)
            # tail: outproj(3, rc0)'s a2a landed mid-batch; rc1's flies
            # under rc0's matmuls
            for rc in range(2):
                for n in range(2):
                    outproj_unit(3, rc, n)
    nc.compile()
    return nc


def prep_inputs(x, context, Wqkv, bqkv, Wout, bout):
    """Host-side sharding: returns in_maps for the 8 cores."""
    x = np.asarray(x, np.float32)
    context = np.asarray(context, np.float32)
    Wqkv = np.asarray(Wqkv, np.float32)
    bqkv = np.asarray(bqkv, np.float32)
    Wout = np.asarray(Wout, np.float32)
    bout = np.asarray(bout, np.float32)

    xT = np.ascontiguousarray(x.reshape(BN, DIM).T).astype(BF16)
    woutT = np.ascontiguousarray(Wout.T).astype(BF16)
    boutb = np.broadcast_to(bout, (128, DIM)).astype(np.float32).copy()

    in_maps = []
    for c in range(NC):
        h0 = c * HPC
        # feature order: [q_h0 | q_h1] then [k_h0 | k_h1]
        wq = Wqkv[h0 * HD:(h0 + HPC) * HD]
        wk = Wqkv[DIM + h0 * HD:DIM + (h0 + HPC) * HD]
        wqkT = np.ascontiguousarray(
            np.concatenate([wq, wk], axis=0).T).astype(BF16)
        bq = np.concatenate([bqkv[h0 * HD:(h0 + HPC) * HD],
                             bqkv[DIM + h0 * HD:DIM + (h0 + HPC) * HD]])
        bq = bq.reshape(2 * 128, 1).astype(np.float32)
        ctxa = np.ones((B, HPC, 128, NKC, CW), np.float32)
        for h in range(HPC):
            g = h0 + h
            arr = context[:, :, g * HD:(g + 1) * HD].reshape(B, NKC, 128, HD)
            ctxa[:, h, :, :, :HD] = arr.transpose(0, 2, 1, 3)
        in_maps.append({
            "xT": xT,
            "wqkT": wqkT,
            "bqk": bq,
            "ctxa": ctxa.reshape(B, HPC, 128, NKC * CW).astype(BF16),
            "woutT": woutT,
            "boutb": boutb,
        })
    return in_maps


_NC_CACHE = None


def _get_nc():
    global _NC_CACHE
    if _NC_CACHE is None:
        _NC_CACHE = build()
    return _NC_CACHE


def run(in_maps, trace=False):
    nc = _get_nc()
    res = run_bass_kernel_spmd(nc, in_maps, core_ids=list(range(NC)),
                               trace=trace)
    # core c's out = [B, 2, 128, DIM]: stripe (b, hf) holds batch-b rows
    # [hf*1024 + c*128 : hf*1024 + (c+1)*128]
    full = np.empty((B, N, DIM), np.float32)
    for c in range(NC):
        o = np.asarray(res.results[c]["out"]).reshape(B, 2, 128, DIM)
        for hf in range(2):
            full[:, hf * 1024 + c * 128:hf * 1024 + (c + 1) * 128, :] = \
                o[:, hf]
    return full, res


def kernel(x, context, Wqkv, bqkv, Wout, bout):
    in_maps = prep_inputs(x, context, Wqkv, bqkv, Wout, bout)
    out, _ = run(in_maps, trace=False)
    return out
